# revision 20
# baseline (speedup 1.0000x reference)
"""DimensionalConsistencyLoss on 8 Trainium2 NeuronCores.

The loss touches only gathered rows of the [100000, 512] f32 table: 8192
pos/neg constraints read one row each (sparsity term + target element), 2048
neu constraints read one element each.

Per core (1/8 of the pos/neg constraints = 1024 row slots = 8 columns of
128, dealt by the host into CLASS-PURE columns: 0-3 pos, 4-7 neg):
  - 8 row-gather indirect DMAs.  The DMA_INDIRECT1D ISA allows one index
    per channel, so 128 rows/instruction is the hardware maximum, and the
    SWDGE ring processes ~128 descriptors / 1.4us regardless of payload -
    the 2MB/core drain through that ring is the critical path.
  - Scalar: activation(Abs, accum_out) per landed column writes the row
    |.| sums straight into the output tile.
  - DVE: extracts t per column via (ramp==dim)*row with accumulate,
    also straight into the output tile.
  - out = [t (8 cols) | rowsum (8 cols)] -> one [128,16] DMA out; no
    completion wait (walrus's end-of-NEFF queue drain covers the store).

The host epilogue turns the 8K extracted scalars into the loss (the
per-slot sign-loss algebra plus the linear c*rowsum sparsity term) and
folds in the neu class partial (sum of 2|emb[id,dim]|, 8KB of reads):
gathering those 2048 single elements on-device would cost two more full
ring slots (~2.8us) plus a ~2.7us completion-latency tail for 0.04% of
the memory traffic.  All memory-bound work - the 2MB/core row gather,
|.| rowsums and target-element extraction - stays on device.

No nc.Block(): engine streams are straight-line, so the block entry/exit
barriers and the explicit semaphore range-clear disappear; walrus's own
end-of-NEFF drain + barrier + semaphore-file clear provides re-runnability.
"""

import numpy as np

import concourse.bacc as bacc
import concourse.bass as bass
import concourse.mybir as mybir
from concourse.bass_utils import run_bass_kernel_spmd

P = 128
VOCAB = 100000
DIM = 512
N_POS = 4096
N_NEG = 4096
N_NEU = 2048
N_ALL = N_POS + N_NEG + N_NEU
N_CORES = 8

RCOLS = (N_POS + N_NEG) // N_CORES // P    # 8 row-gather columns (pos/neg)
OUTW = 2 * RCOLS                           # 16: A (8) | rowsum (8)
RDFW = DIM + RCOLS                         # ramp | dims input width

CONSISTENCY_WEIGHT = 0.5
SPARSITY_WEIGHT = 0.1
C_SP = SPARSITY_WEIGHT / (DIM - 1)

F32 = mybir.dt.float32
I32 = mybir.dt.int32
OP = mybir.AluOpType
AF = mybir.ActivationFunctionType

_nc_cache = None


def _build_program():
    global _nc_cache
    if _nc_cache is not None:
        return _nc_cache

    nc = bacc.Bacc(
        "TRN2", target_bir_lowering=False, debug=False, num_devices=N_CORES,
        num_swdge_queues=1,
    )
    emb = nc.dram_tensor("emb", [VOCAB, DIM], F32, kind="ExternalInput")
    idx_d = nc.dram_tensor("idx", [P, RCOLS], I32, kind="ExternalInput")
    rdf_d = nc.dram_tensor("rdf", [P, RDFW], F32, kind="ExternalInput")
    out_d = nc.dram_tensor("out", [P, OUTW], F32, kind="ExternalOutput")

    from contextlib import ExitStack

    with ExitStack() as ctx:
        sb = lambda name, shape, dt=F32: ctx.enter_context(
            nc.sbuf_tensor(name, shape, dt)
        )
        idx_sb = sb("idx_sb", [P, RCOLS], I32)
        rdf_sb = sb("rdf_sb", [P, RDFW])
        rows = sb("rows", [P, RCOLS, DIM])
        s_act = sb("s_act", [P, DIM])
        s_dve = sb("s_dve", [P, DIM])
        out_sb = sb("out_sb", [P, OUTW])
        sem = lambda name: ctx.enter_context(nc.semaphore(name))
        idx0_s, idx_s, rdf_s = sem("idx0_s"), sem("idx_s"), sem("rdf_s")
        rg = [sem(f"rg{j}") for j in range(RCOLS + 1)]
        dv, sc, io2 = sem("dv"), sem("sc"), sem("io2")

        # ---- SP: index loads now, output store at the end.  The first two
        # index columns go in a tiny DMA of their own so the first row
        # gather starts ~0.7us earlier (completion latency scales with
        # size; the gather chain head is on the critical path).
        nc.sync.dma_start(idx_sb[:, 0:2], idx_d[:, 0:2]).then_inc(idx0_s, 16)
        nc.sync.dma_start(idx_sb[:, 2:], idx_d[:, 2:]).then_inc(idx_s, 16)

        # ---- Scalar: ramp|dims load on the Activation HWDGE queue
        # (parallel with SP's loads), then per-column |row| sums.
        nc.scalar.dma_start(rdf_sb[:, :], rdf_d[:, :]).then_inc(rdf_s, 16)

        # ---- GpSimd: the SWDGE row gathers.
        nc.gpsimd.wait_ge(idx0_s, 16)
        for j in range(RCOLS):
            if j == 2:
                nc.gpsimd.wait_ge(idx_s, 16)
            nc.gpsimd.indirect_dma_start(
                out=rows[:, j, :],
                out_offset=None,
                in_=emb[:, :],
                in_offset=bass.IndirectOffsetOnAxis(
                    ap=idx_sb[:, j : j + 1], axis=0
                ),
            ).then_inc(rg[j], 16)

        # ---- Scalar: |row| sums, accumulated straight into the out tile.
        for j in range(RCOLS):
            nc.scalar.wait_ge(rg[j], 16)
            nc.scalar.activation(
                s_act[:, :], rows[:, j, :], AF.Abs,
                accum_out=out_sb[:, RCOLS + j : RCOLS + j + 1],
            ).then_inc(sc, 1)

        # ---- DVE: extract t per row column, accumulated straight into the
        # out tile (the per-slot sign-loss algebra on these 8K scalars is
        # folded into the host epilogue with the other partials).
        nc.vector.wait_ge(rdf_s, 16)
        for j in range(RCOLS):
            nc.vector.wait_ge(rg[j], 16)
            nc.vector.scalar_tensor_tensor(
                out=s_dve[:, :],
                in0=rdf_sb[:, 0:DIM],
                scalar=rdf_sb[:, DIM + j : DIM + j + 1],
                in1=rows[:, j, :],
                op0=OP.is_equal,
                op1=OP.mult,
                accum_out=out_sb[:, j : j + 1],
            ).then_inc(dv, 1)

        # ---- SP: store once all t's and rowsums are done.  No completion
        # wait: walrus's end-of-NEFF queue drain covers the in-flight store
        # before the final barrier/halt.
        nc.sync.wait_ge(dv, RCOLS)
        nc.sync.wait_ge(sc, RCOLS)
        nc.sync.dma_start(out_d[:, :], out_sb[:, :]).then_inc(io2, 16)

    nc.compile()
    _nc_cache = nc
    return nc


def _sorted_block(ids, dims, blocks):
    """Sort (ids, dims) by id and split into `blocks` contiguous chunks."""
    o = np.argsort(ids, kind="stable")
    si, sd = ids[o], dims[o]
    n = len(ids) // blocks
    return [(si[c * n : (c + 1) * n], sd[c * n : (c + 1) * n])
            for c in range(blocks)]


def _deal(pos_ids, pos_dims, neg_ids, neg_dims):
    """Deal pos/neg constraints into per-core class-pure column tables.

    Ids are sorted ascending and dealt in contiguous blocks, so core c's
    gathers walk one narrow ascending slice of the vocab and the 8 cores
    touch disjoint regions.  The loss is a sum over slots, so any
    permutation is valid.

    Returns per-core (idx [128,8] int32 row ids;
                      rdf [128,520] f32: iota ramp | per-slot dims).
    """
    pos = _sorted_block(pos_ids, pos_dims, N_CORES)
    neg = _sorted_block(neg_ids, neg_dims, N_CORES)
    idx_all, rdf_all = [], []
    for c in range(N_CORES):
        (pid, pdm), (nid, ndm) = pos[c], neg[c]
        idx = np.concatenate([pid, nid]).reshape(RCOLS, P).T.astype(np.int32)
        rdf = np.empty((P, RDFW), np.float32)
        rdf[:, 0:DIM] = np.arange(DIM, dtype=np.float32)[None, :]
        rdf[:, DIM:] = np.concatenate([pdm, ndm]).reshape(RCOLS, P).T
        idx_all.append(np.ascontiguousarray(idx))
        rdf_all.append(np.ascontiguousarray(rdf))
    return idx_all, rdf_all


def _make_in_maps(emb, pos_ids, pos_dims, neg_ids, neg_dims, neu_ids, neu_dims):
    idx, rdf = _deal(pos_ids, pos_dims, neg_ids, neg_dims)
    return [{"emb": emb, "idx": idx[c], "rdf": rdf[c]} for c in range(N_CORES)]


def _neu_partial(emb, neu_ids, neu_dims):
    """Host partial for the neu class: sum of 2|emb[id, dim]|."""
    return 2.0 * np.abs(emb[neu_ids, neu_dims].astype(np.float64)).sum()


def _finish(results, neu_part):
    """Host epilogue: per-slot sign loss from the extracted t's, plus the
    linear sparsity rowsum term, summed over cores and scaled."""
    total = float(neu_part)
    for r in results:
        o = r["out"].astype(np.float64)
        t, rowsum = o[:, 0:RCOLS], o[:, RCOLS:]
        a = np.abs(t)
        m = np.empty_like(t)
        m[:, 0:4] = t[:, 0:4] <= 0          # pos: wrong sign is t<=0
        m[:, 4:8] = t[:, 4:8] >= 0          # neg: wrong sign is t>=0
        A = m * ((1.0 + SPARSITY_WEIGHT) * a + SPARSITY_WEIGHT) - (
            SPARSITY_WEIGHT + C_SP) * a
        total += A.sum() + C_SP * rowsum.sum()
    return np.asarray(total * CONSISTENCY_WEIGHT / N_ALL, dtype=np.float32)


def kernel(**inputs):
    emb = np.ascontiguousarray(np.asarray(inputs["embeddings"], dtype=np.float32))
    ids = {
        k: np.asarray(inputs[k]).astype(np.int64)
        for k in ("pos_ids", "pos_dims", "neg_ids", "neg_dims", "neu_ids", "neu_dims")
    }
    nc = _build_program()
    in_maps = _make_in_maps(
        emb, ids["pos_ids"], ids["pos_dims"], ids["neg_ids"], ids["neg_dims"],
        ids["neu_ids"], ids["neu_dims"],
    )
    res = run_bass_kernel_spmd(nc, in_maps, list(range(N_CORES)))
    return _finish(res.results, _neu_partial(emb, ids["neu_ids"], ids["neu_dims"]))
